# revision 53
# baseline (speedup 1.0000x reference)
"""Bahdanau-attention scores kernel for Trainium2 (8 NeuronCores, SPMD).

Computation (per batch row b):
    pre[s, k] = hidden[b] @ Wh + enc[b, s] @ We + b_attn       (S=1024, E=K=1024)
    scores[s] = tanh(pre[s, :]) @ v
    out[b]    = softmax(where(mask[b]==0, -1e10, scores))      over s

Sharding: data-parallel over batch B=64 -> 8 batches per core; weights
replicated. No collectives.

Per-core structure (fp8 DoubleRow main matmul, bf16 elsewhere):
  - enc pipeline per batch: DRAM->DRAM SWDGE cast f32->fp8e4 (8 chunks
    over the 8 SWDGE queues, ~13us: ~300 GB/s read-side; the 1MB fp8
    write hides inside the 4MB read), then ONE full-batch DRAM->SBUF xbar
    transpose of the fp8 pairs as u16 (issue ~4us + drain ~5us):
    encT8[p, et, s] u16 = (enc[s, et*256+2p], enc[s, et*256+2p+1]) --
    exactly the DoubleRow rhs pairing.
  - THE governing constraint: Tile fences every xbar transpose against
    ALL in-flight DMAs on every queue (SWDGE + both HWDGE rings, both
    directions).  All DMA time is therefore strictly additive around each
    transpose; the steady-state floor is cast(13) + tp(9) ~= 22us/batch,
    above the PE's ~17us.  Consequences baked into the schedule:
      * ONE transpose per batch (half-splits pay a second ~5us fence hop
        and interleave with cast groups -- measured worse twice);
      * per-iteration emission: [batch-b MM loop] -> [tp(b+1)] ->
        [cast(b+2)], so exactly one tp-block and one cast-block alternate
        per fence cycle; tp first, because its dependency (cast(b+1))
        finished last iteration, so encT(b+1) lands with slack and run-to-
        run jitter stops re-throttling HAM at batch starts;
      * MMs are emitted BEFORE the next tp: readers of a transpose-written
        tile conservatively wait on the LAST transpose emitted before
        them (shared HWDGE semaphore pool), so a tp emitted before the
        MMs that consume the PREVIOUS tp adds a spurious ~10us stall;
      * alternatives measured and rejected: HWDGE loads + DVE cast +
        bounce store (SBUF->DRAM stores cap at ~87 GB/s on every path,
        and HWDGE rings allow only ~3 in-flight DMAs with ~143 GB/s per
        DMA); strided truncated-bf16 loads (DMA needs a contiguous last
        dim); DVE 32x32 stream-transpose (32-lane limited, ~12us/batch).
  - one DRAM bounce tensor per batch so coarse DRAM-range tracking never
    serializes different batches' casts/transposes.
  - main MM: pre[k, s] = sum_et lhsT(w8) @ rhs(encT8), DoubleRow, one
    LDWEIGHTS per (et, kt) serving both s-halves; ~216ns/MM warm.
  - ScalarE: tanh(psum/64 + (hidden@Wh + b_attn)[k]) -> SBUF bf16
  - hidden@Wh (hp) is interleaved per-kt into batch 0's loop (wh loaded as
    per-kt chunks on the scalar ring) so it doesn't sit at the PE FIFO
    head blocking the first main MMs behind a 2MB weight load.
  - v-dot: 4 col-tiled PE matmuls (tile_position=(0,32q)), lag FOUR
    k-tiles behind the main MMs (ACT falls ~1.5 groups behind the PE by
    batch end; lag 2 stalled the PE 0.5-1.5us per batch).  Last 4 k-tiles
    carried into the next batch, one k-tile per MM group, all 4 quarter
    MMs before the 4 flat4 copies (interleaving MMs and copies created
    false column-range WARs = 3x850ns PE bubbles).  tanh pool is 6 deep so
    the carried v-dots' reads never make the next batch's tanh wait.
  - softmax in chunks: batches 0-5 during iter 7, batch 6 right after its
    carry, 7 alone on the tail.  scores is pre-filled with (mask-1)*1e10;
    gathers flat4->scores are SWDGE accumulate-adds spread over 4 queues
    (on the scalar/ACT ring they blocked the tanh stream: -29us).  The
    final chunk recomputes rows 0..7 (DVE/ACT partition windows must
    start at partition 0; the recompute is idempotent and lane-parallel).

Sync note: this walrus build encodes at most ONE semaphore wait per
instruction; _split_multi_waits() rewrites Tile's multi-wait instructions
into NoOp(wait) chains on the same engine.
"""

import sys

if "/opt/trn_rl_repo" not in sys.path:
    sys.path.insert(0, "/opt/trn_rl_repo")

from contextlib import ExitStack

import numpy as np

B, S, E, K = 64, 1024, 1024, 1024  # E = 2*ENC_HID, K = DEC_HID
NCORES = 8
BL = B // NCORES  # batches per core
NEG = -1e10
WSCALE = 64.0     # We quantization scale into E4M3 range

ET2 = E // 256  # 4 DoubleRow e-tiles (256-deep contraction each)
KT = K // 128   # 8 k-tiles
ST = S // 128   # 8 s-tiles
NB = 512        # matmul free-dim block (one s-half)
SB = S // NB    # 2 s-halves
VLAG = 4        # v-dot lag in k-tiles behind the main MMs

_CACHE = {}


def _build_bass(strip=True):
    from concourse import bass, mybir, tile

    f32 = mybir.dt.float32
    bf16 = mybir.dt.bfloat16
    f8 = mybir.dt.float8e4
    u16 = mybir.dt.uint16
    i32 = mybir.dt.int32
    Tanh = mybir.ActivationFunctionType.Tanh
    Exp = mybir.ActivationFunctionType.Exp
    Alu = mybir.AluOpType
    Ax = mybir.AxisListType
    DR = mybir.MatmulPerfMode.DoubleRow

    nc = bass.Bass()

    enc_d = nc.declare_dram_parameter("encoder_outputs", [BL, S, E], f32, isOutput=False)
    # one bounce tensor per batch: no shared-tensor dep tracking.
    # batch 0 additionally gets per-half tensors so its first transpose
    # only RAW-depends on half the casts (startup fill).
    enc8_d = [nc.dram_tensor(f"enc8_{b}", [S, E], f8) for b in range(BL)]
    enc8_b0h = [nc.dram_tensor(f"enc8_0h{h}", [NB, E], f8) for h in range(SB)]
    mask_d = nc.declare_dram_parameter("mask", [BL, S], i32, isOutput=False)
    wh_d = nc.declare_dram_parameter("wh_pack", [128, KT, K], bf16, isOutput=False)
    w8_d = nc.declare_dram_parameter("w8_pack", [128, ET2, 2, K], f8, isOutput=False)
    hT_d = nc.declare_dram_parameter("hT_pack", [128, KT * BL], bf16, isOutput=False)
    b_d = nc.declare_dram_parameter("b_pack", [1, K], bf16, isOutput=False)
    v_d = nc.declare_dram_parameter("v_pack", [128, KT], bf16, isOutput=False)
    out_d = nc.declare_dram_parameter("out", [BL, S], f32, isOutput=True)

    with tile.TileContext(nc) as tc, ExitStack() as ctx:
        const = ctx.enter_context(tc.tile_pool(name="const", bufs=1))
        tp_pool = ctx.enter_context(tc.tile_pool(name="encT", bufs=3))
        tanh_pool = ctx.enter_context(tc.tile_pool(name="tanh", bufs=6))
        pre_ps = ctx.enter_context(tc.tile_pool(name="pre_ps", bufs=3, space="PSUM"))
        sc_ps = ctx.enter_context(tc.tile_pool(name="sc_ps", bufs=1, space="PSUM"))
        fin = ctx.enter_context(tc.tile_pool(name="fin", bufs=1))

        # ---- weights (host-packed) ----
        # w8 gates the first main matmul: first on the sync ring, in two
        # DMAs so they overlap (per-DMA rate is ~143 GB/s)
        w8 = const.tile([128, ET2, 2, K], f8)
        nc.sync.dma_start(w8[:, :2], w8_d[:, :2])
        nc.sync.dma_start(w8[:, 2:], w8_d[:, 2:])
        hT_bf = const.tile([128, KT * BL], bf16)
        nc.sync.dma_start(hT_bf[:], hT_d[:])
        b_attn_bf = const.tile([1, K], bf16)
        nc.sync.dma_start(b_attn_bf[:], b_d[:])
        v_bf = const.tile([128, KT], bf16)
        nc.sync.dma_start(v_bf[:], v_d[:])
        # wh per-kt chunks on the scalar ring (repacked kt-major on host)
        wh_bf = const.tile([128, KT, KT, 128], bf16)  # [p, kt, dt, c]

        ones_bf = const.tile([1, BL], bf16)
        nc.vector.memset(ones_bf[:], 1.0)
        scratch = const.tile([1, K], bf16)

        def dummy_dma():
            # 2KB no-op DMA: pads the HWDGE/SWDGE semaphore-pool rotation
            # (9 sems, 9 real DMAs per iteration) so a transpose's semaphore
            # is reused by THIS instantly-completing dummy instead of the
            # NEXT iteration's transpose -- readers of a semaphore are
            # conservatively bumped to its next user's count, which was
            # making every batch's first MMs wait on the FOLLOWING batch's
            # transpose (~2.5us/batch, ~9us at b1, ~17us at b0)
            nc.sync.dma_start(scratch[:], b_d[:])

        def stage_cast(b):
            """f32 -> fp8 DRAM->DRAM SWDGE cast of one batch, in 8 chunks
            round-robined over the 8 SWDGE queues (the ~300 GB/s read side
            is the limiter; the 1MB fp8 write hides inside the 4MB read)."""
            for st in range(ST):
                nc.gpsimd.dma_start(
                    enc8_d[b][st * 128:(st + 1) * 128, :],
                    enc_d[b, st * 128:(st + 1) * 128, :])

        def stage_tp(b):
            """DRAM->SBUF u16-pair xbar transpose of the fp8 bounce:
            encT8 u16[p, et, s] = fp8 pair
            (enc[b, s, et*256+2p], enc[b, s, et*256+2p+1]).
            Every transpose is a full DMA-system barrier (Tile fences it
            against ALL in-flight DMAs, both directions), so the schedule
            strictly alternates cast(batch) / tp(batch) — ONE fence cycle
            per batch; splitting into halves costs a second ~5us hop."""
            eh = tp_pool.tile([128, ET2, S], u16, tag="encT",
                              name=f"encT_{b}")
            # ONE full-size tp per batch: half-splitting was tried twice and
            # loses ~2-5us/batch — the halves interleave with cast groups
            # into two fence cycles per batch
            nc.sync.dma_start(eh[:], enc8_d[b][:].bitcast(u16),
                              transpose=True)
            return eh

        def stage_cast_b0h(h):
            for st in range(ST // SB):
                nc.gpsimd.dma_start(
                    enc8_b0h[h][st * 128:(st + 1) * 128, :],
                    enc_d[0, h * NB + st * 128:h * NB + (st + 1) * 128, :])

        def stage_tp_b0h(h):
            ehh = tp_pool.tile([128, ET2, NB], u16, tag="encTh",
                               name=f"encT0h{h}", bufs=2)
            nc.sync.dma_start(ehh[:], enc8_b0h[h][:].bitcast(u16),
                              transpose=True)
            return ehh

        def rhs_view(eh, f8dt):
            return eh[:].bitcast(f8dt).rearrange("p et (s j) -> p et j s", j=2)

        hpb = const.tile([128, KT * BL], f32)  # col = kt*BL + b

        def emit_hp_kt(kt):
            # h_proj[k, b] = sum_d Wh[d, k]*hidden[b, d] + b_attn[k],
            # one k-tile at a time, interleaved into batch 0's MM stream
            hp_ps = pre_ps.tile([128, NB], f32, tag="pre", name=f"hp_ps{kt}",
                                bufs=7)
            for dt in range(KT):
                nc.tensor.matmul(
                    hp_ps[:, :BL],
                    wh_bf[:, kt, dt, :],
                    hT_bf[:, dt * BL:(dt + 1) * BL],
                    start=(dt == 0),
                    stop=False,
                )
            nc.tensor.matmul(
                hp_ps[:, :BL],
                b_attn_bf[:, kt * 128:(kt + 1) * 128],
                ones_bf[:],
                start=False,
                stop=True,
            )
            # ACT, not DVE: the DVE is busy with next-batch casts at b0
            # start and the first tanh would block behind them (gpsimd
            # cannot read PSUM)
            nc.scalar.copy(hpb[:, kt * BL:(kt + 1) * BL], hp_ps[:, :BL])

        # scores accumulate on PSUM rows 32q (col-group q = s-quarter q);
        # staged in flat4 rows 32q on SBUF, gathered to [b, S] chunks
        flat4 = fin.tile([97, BL * 256], f32)

        # softmax state.  scores is pre-filled with (mask-1)*1e10 at mask
        # prep; the gathers then ACCUMULATE flat4 into it (SWDGE add), so
        # the masked add costs nothing on the tail chain.
        mask_i = fin.tile([BL, S], i32)
        mask_f = fin.tile([BL, S], f32)
        scores = fin.tile([BL, S], f32)
        negmax = fin.tile([BL, 1], f32)
        expv = fin.tile([BL, S], f32)
        rowsum = fin.tile([BL, 1], f32)
        recip = fin.tile([BL, 1], f32)
        outf = fin.tile([BL, S], f32)

        def emit_mask_prep():
            nc.scalar.dma_start(mask_i[:], mask_d[:])
            nc.vector.tensor_copy(mask_f[:], mask_i[:])
            nc.vector.tensor_scalar(
                scores[:], mask_f[:], -NEG, NEG, Alu.mult, Alu.add)

        def emit_gathers(b0, b1):
            # gather flat4 quarter-rows into [b, s] layout; SWDGE spreads
            # the 4 tiny SBUF->SBUF copies over 4 queues (parallel) — only
            # emitted after the last transpose, so no xbar fence stalls
            for q in range(4):
                nc.gpsimd.dma_start(
                    scores[b0:b1, q * 256:(q + 1) * 256],
                    flat4[32 * q:32 * q + 1, b0 * 256:b1 * 256],
                    accum_op=Alu.add)

        def emit_softmax_chunk(b0, b1, gather=True):
            if gather:
                emit_gathers(b0, b1)
            # compute ops always span [0, b1): DVE/ACT partition windows
            # must start at 0 (quadrant alignment); recomputing the already
            # finished low batches is idempotent and partition-parallel
            nc.vector.tensor_reduce(
                negmax[:b1], scores[:b1, :], Ax.X, Alu.max, negate=True)
            nc.scalar.activation(
                expv[:b1, :], scores[:b1, :], Exp, bias=negmax[:b1],
                accum_out=rowsum[:b1])
            nc.vector.reciprocal(recip[:b1], rowsum[:b1])
            nc.vector.tensor_scalar_mul(
                outf[:b1, :], expv[:b1, :], recip[:b1])
            nc.scalar.dma_start(out_d[b0:b1, :], outf[b0:b1, :])

        # ---- prologue ----
        # wh chunks on the scalar ring (free until the gathers at the end)
        for ck in range(KT):
            # wh_d dims are [p, kt, (dt c)] after the host kt-major repack
            nc.scalar.dma_start(wh_bf[:, ck], wh_d[:, ck])
        encTs = {}

        # batch 0 runs at HALF granularity so the PE starts ~23us earlier:
        # [cast b0h0, tp b0h0] -> sb0 MM pass (emitted HERE so its
        # conservative last-transpose wait is tp(b0h0), not a later tp) ->
        # [tp b0h1, cast b1] -> sb1 pass -> [tp b1, cast b2] -> main loop
        stage_cast_b0h(0)
        encT0h = [stage_tp_b0h(0)]
        stage_cast_b0h(1)
        for _ in range(5):
            dummy_dma()

        # scores PSUM: ONE bank, halves alternated by batch parity
        scband = sc_ps.tile([128, 2, 256], f32, tag="sc", name="scband")

        def scq(b, q):
            return scband[32 * q:32 * q + 1, b % 2, :]

        carry = []  # [(b_prev, kt, th_tile, col_off, qs)] not yet emitted

        def emit_vdots(b, kt, th, col_off, qs, stop):
            for q in qs:
                nc.tensor.matmul(
                    scq(b, q),
                    v_bf[:, kt:kt + 1],
                    th[:, col_off + (q - qs[0]) * 256:
                       col_off + (q - qs[0] + 1) * 256],
                    start=(kt == 0), stop=stop,
                    tile_position=(0, 32 * q))
            if stop:
                # all MMs above before any copy: interleaving creates false
                # column-range WARs that bubble the PE ~850ns per quarter
                for q in qs:
                    nc.vector.tensor_copy(
                        flat4[32 * q:32 * q + 1, b * 256:(b + 1) * 256],
                        scq(b, q))

        def emit_carry(budget):
            while carry and budget > 0:
                b_p, kt, th, col_off, qs = carry.pop(0)
                emit_vdots(b_p, kt, th, col_off, qs, stop=(kt == KT - 1))
                budget -= 1

        def emit_b0_pass(sb, eh):
            rh = rhs_view(eh, f8)
            for kt in range(KT):
                pre = pre_ps.tile([128, NB], f32, tag="pre",
                                  name="preh", bufs=7)
                for et in range(ET2):
                    nc.tensor.matmul(
                        pre[:],
                        w8[:, et, :, kt * 128:(kt + 1) * 128],
                        rh[:, et, :, :],
                        start=(et == 0),
                        stop=(et == ET2 - 1),
                        perf_mode=DR,
                    )
                if sb == 0:
                    emit_hp_kt(kt)
                th = tanh_pool.tile([128, NB], bf16, tag="thh",
                                    name="thh", bufs=5)
                nc.scalar.activation(
                    th[:], pre[:], Tanh,
                    bias=hpb[:, kt * BL:kt * BL + 1],
                    scale=1.0 / WSCALE,
                )
                if kt >= 1:
                    emit_vdots(0, kt - 1, b0_ths[(sb, kt - 1)], 0,
                               (2 * sb, 2 * sb + 1), stop=False)
                b0_ths[(sb, kt)] = th
            carry.append((0, KT - 1, b0_ths[(sb, KT - 1)], 0,
                          (2 * sb, 2 * sb + 1)))

        b0_ths = {}
        emit_b0_pass(0, encT0h[0])
        encT0h.append(stage_tp_b0h(1))
        stage_cast(1)
        dummy_dma()
        emit_b0_pass(1, encT0h[1])
        encTs[1] = stage_tp(1)
        stage_cast(2)
        dummy_dma()

        # ---- main loop over local batches (software-pipelined) ----
        for b in range(1, BL):
            if b == 1:
                emit_mask_prep()

            rhv = rhs_view(encTs.pop(b), f8)
            ths = {}
            for kt in range(KT):
                pres = [pre_ps.tile([128, NB], f32, tag="pre",
                                    name=f"pre{sb}", bufs=7)
                        for sb in range(SB)]
                for et in range(ET2):  # one LDWEIGHTS serves both sb
                    for sb in range(SB):
                        nc.tensor.matmul(
                            pres[sb][:],
                            w8[:, et, :, kt * 128:(kt + 1) * 128],
                            rhv[:, et, :, sb * NB:(sb + 1) * NB],
                            start=(et == 0),
                            stop=(et == ET2 - 1),
                            perf_mode=DR,
                        )
                if b == 0:
                    # hidden@Wh one k-tile at a time, off the startup
                    # critical path (wh streams in per-kt chunks)
                    emit_hp_kt(kt)
                elif kt < VLAG:
                    emit_carry(1)
                th = tanh_pool.tile([128, SB * NB], bf16, tag="tanh",
                                    bufs=6)
                for sb in range(SB):
                    nc.scalar.activation(
                        th[:, sb * NB:(sb + 1) * NB], pres[sb][:], Tanh,
                        bias=hpb[:, kt * BL + b:kt * BL + b + 1],
                        scale=1.0 / WSCALE,
                    )
                ths[kt] = th
                if kt >= VLAG:
                    emit_vdots(b, kt - VLAG, ths[kt - VLAG], 0,
                               (0, 1, 2, 3), stop=False)

            for kt in range(KT - VLAG, KT):
                carry.append((b, kt, ths[kt], 0, (0, 1, 2, 3)))
            # stage the next batches AFTER this batch's MMs (readers of a
            # transpose-written tile conservatively wait on the last
            # transpose emitted before them); tp(b+1) first: its cast
            # finished last iteration, so encT(b+1) lands early with slack
            if b + 1 < BL:
                encTs[b + 1] = stage_tp(b + 1)
            if b + 2 < BL:
                stage_cast(b + 2)
                dummy_dma()
            if b == 7:
                emit_softmax_chunk(0, 6)

        emit_gathers(6, 7)  # b6's flat4 is final; runs during b7's tail MMs
        emit_carry(len(carry))
        emit_gathers(7, 8)
        emit_softmax_chunk(6, 8, gather=False)

    if strip:
        _split_multi_waits(nc, mybir)
    return nc


def _split_multi_waits(nc, mybir):
    """Move extra semaphore waits onto standalone NoOps on the same engine.

    This walrus build encodes at most one sync-wait command per instruction,
    but Tile emits instructions with several (cross-engine RAW + WAR + DMA
    queue ordering). A NoOp carrying one wait, placed immediately before the
    instruction in the same engine's stream, is semantically identical: the
    engine's sequencer blocks on the NoOp's wait before dispatching the real
    instruction.
    """
    n = 0
    for fn in nc.m.functions:
        for blk in fn.blocks:
            insts = blk.instructions
            new = []
            changed = False
            for inst in insts:
                si = inst.sync_info
                if si is not None and si.on_wait and len(si.on_wait) > 1:
                    for w in list(si.on_wait)[:-1]:
                        n += 1
                        new.append(mybir.InstNoOp(
                            name=f"{inst.name}-sw{n}",
                            engine=inst.engine,
                            text_hint="split_wait",
                            bass_nofuse=True,
                            sync_info=mybir.SyncInfo(
                                on_wait=[w], on_update=[]),
                        ))
                    inst.sync_info = mybir.SyncInfo(
                        on_wait=[list(si.on_wait)[-1]],
                        on_update=list(si.on_update or []))
                    changed = True
                new.append(inst)
            if changed:
                blk.instructions = new


def get_nc(strip=True):
    key = ("nc", strip)
    if key not in _CACHE:
        _CACHE[key] = _build_bass(strip)
    return _CACHE[key]


def make_in_maps(hidden, encoder_outputs, mask, W_attn, b_attn, v):
    import ml_dtypes

    bf16 = ml_dtypes.bfloat16
    f8 = ml_dtypes.float8_e4m3

    W_attn = np.asarray(W_attn, dtype=np.float32)
    Wh, We = W_attn[:K], W_attn[K:]
    # wh_pack[p, kt, dt, c] = Wh[dt*128 + p, kt*128 + c]  (kt-major chunks)
    wh_pack = np.ascontiguousarray(
        Wh.reshape(KT, 128, KT, 128).transpose(1, 2, 0, 3).astype(bf16))
    # w8_pack[p, et, j, k] = 64 * We[et*256 + 2p + j, k]
    w8_pack = np.ascontiguousarray(
        (We * WSCALE).reshape(ET2, 128, 2, K).transpose(1, 0, 2, 3).astype(f8))
    b_pack = np.ascontiguousarray(
        np.asarray(b_attn, dtype=np.float32).reshape(1, K).astype(bf16))
    # v_pack[p, kt] = v[kt*128 + p]
    v_pack = np.ascontiguousarray(
        np.asarray(v, dtype=np.float32).reshape(KT, 128).T.astype(bf16))
    hidden = np.asarray(hidden, dtype=np.float32)

    in_maps = []
    for c in range(NCORES):
        sl = slice(c * BL, (c + 1) * BL)
        # hT_pack[p, dt*BL + b] = hidden[b, dt*128 + p]
        hT_pack = np.ascontiguousarray(
            hidden[sl].T.reshape(KT, 128, BL).transpose(1, 0, 2)
            .reshape(128, KT * BL).astype(bf16))
        in_maps.append({
            "encoder_outputs": np.ascontiguousarray(encoder_outputs[sl]),
            "mask": np.ascontiguousarray(np.asarray(mask[sl], dtype=np.int32)),
            "wh_pack": wh_pack,
            "w8_pack": w8_pack,
            "hT_pack": hT_pack,
            "b_pack": b_pack,
            "v_pack": v_pack,
        })
    return in_maps


def kernel(hidden, encoder_outputs, mask, W_attn, b_attn, v):
    from concourse.bass_utils import run_bass_kernel_spmd

    nc = get_nc()
    in_maps = make_in_maps(hidden, encoder_outputs, mask, W_attn, b_attn, v)
    res = run_bass_kernel_spmd(nc, in_maps, core_ids=list(range(NCORES)))
    return np.concatenate(
        [np.asarray(res.results[c]["out"], dtype=np.float32) for c in range(NCORES)],
        axis=0,
    )


# revision 55
# speedup vs baseline: 1.0498x; 1.0498x over previous
"""Bahdanau-attention scores kernel for Trainium2 (8 NeuronCores, SPMD).

Computation (per batch row b):
    pre[s, k] = hidden[b] @ Wh + enc[b, s] @ We + b_attn       (S=1024, E=K=1024)
    scores[s] = tanh(pre[s, :]) @ v
    out[b]    = softmax(where(mask[b]==0, -1e10, scores))      over s

Sharding: data-parallel over batch B=64 -> 8 batches per core; weights
replicated. No collectives.

Per-core structure (fp8 DoubleRow main matmul, bf16 elsewhere):
  - enc pipeline per batch: DRAM->DRAM SWDGE cast f32->fp8e4 (8 chunks
    over the 8 SWDGE queues, ~13us: ~300 GB/s read-side; the 1MB fp8
    write hides inside the 4MB read), then ONE full-batch DRAM->SBUF xbar
    transpose of the fp8 pairs as u16 (issue ~4us + drain ~5us):
    encT8[p, et, s] u16 = (enc[s, et*256+2p], enc[s, et*256+2p+1]) --
    exactly the DoubleRow rhs pairing.
  - THE governing constraint: Tile fences every xbar transpose against
    ALL in-flight DMAs on every queue (SWDGE + both HWDGE rings, both
    directions).  All DMA time is therefore strictly additive around each
    transpose; the steady-state floor is cast(13) + tp(9) ~= 22us/batch,
    above the PE's ~17us.  Consequences baked into the schedule:
      * ONE transpose per batch (half-splits pay a second ~5us fence hop
        and interleave with cast groups -- measured worse twice);
      * per-iteration emission: [batch-b MM loop] -> [tp(b+1)] ->
        [cast(b+2)], so exactly one tp-block and one cast-block alternate
        per fence cycle; tp first, because its dependency (cast(b+1))
        finished last iteration, so encT(b+1) lands with slack and run-to-
        run jitter stops re-throttling HAM at batch starts;
      * MMs are emitted BEFORE the next tp: readers of a transpose-written
        tile conservatively wait on the LAST transpose emitted before
        them (shared HWDGE semaphore pool), so a tp emitted before the
        MMs that consume the PREVIOUS tp adds a spurious ~10us stall;
      * alternatives measured and rejected: HWDGE loads + DVE cast +
        bounce store (SBUF->DRAM stores cap at ~87 GB/s on every path,
        and HWDGE rings allow only ~3 in-flight DMAs with ~143 GB/s per
        DMA); strided truncated-bf16 loads (DMA needs a contiguous last
        dim); DVE 32x32 stream-transpose (32-lane limited, ~12us/batch).
  - one DRAM bounce tensor per batch so coarse DRAM-range tracking never
    serializes different batches' casts/transposes.
  - main MM: pre[k, s] = sum_et lhsT(w8) @ rhs(encT8), DoubleRow, one
    LDWEIGHTS per (et, kt) serving both s-halves; ~216ns/MM warm.
  - ScalarE: tanh(psum/64 + (hidden@Wh + b_attn)[k]) -> SBUF bf16
  - hidden@Wh (hp) is interleaved per-kt into batch 0's loop (wh loaded as
    per-kt chunks on the scalar ring) so it doesn't sit at the PE FIFO
    head blocking the first main MMs behind a 2MB weight load.
  - v-dot: 4 col-tiled PE matmuls (tile_position=(0,32q)), lag FOUR
    k-tiles behind the main MMs (ACT falls ~1.5 groups behind the PE by
    batch end; lag 2 stalled the PE 0.5-1.5us per batch).  Last 4 k-tiles
    carried into the next batch, one k-tile per MM group, all 4 quarter
    MMs before the 4 flat4 copies (interleaving MMs and copies created
    false column-range WARs = 3x850ns PE bubbles).  tanh pool is 6 deep so
    the carried v-dots' reads never make the next batch's tanh wait.
  - softmax in chunks: batches 0-5 during iter 7, batch 6 right after its
    carry, 7 alone on the tail.  scores is pre-filled with (mask-1)*1e10;
    gathers flat4->scores are SWDGE accumulate-adds spread over 4 queues
    (on the scalar/ACT ring they blocked the tanh stream: -29us).  The
    final chunk recomputes rows 0..7 (DVE/ACT partition windows must
    start at partition 0; the recompute is idempotent and lane-parallel).

Sync note: this walrus build encodes at most ONE semaphore wait per
instruction; _split_multi_waits() rewrites Tile's multi-wait instructions
into NoOp(wait) chains on the same engine.
"""

import sys

if "/opt/trn_rl_repo" not in sys.path:
    sys.path.insert(0, "/opt/trn_rl_repo")

from contextlib import ExitStack

import numpy as np

B, S, E, K = 64, 1024, 1024, 1024  # E = 2*ENC_HID, K = DEC_HID
NCORES = 8
BL = B // NCORES  # batches per core
NEG = -1e10
WSCALE = 64.0     # We quantization scale into E4M3 range

ET2 = E // 256  # 4 DoubleRow e-tiles (256-deep contraction each)
KT = K // 128   # 8 k-tiles
ST = S // 128   # 8 s-tiles
NB = 512        # matmul free-dim block (one s-half)
SB = S // NB    # 2 s-halves
VLAG = 4        # v-dot lag in k-tiles behind the main MMs

_CACHE = {}


def _build_bass(strip=True):
    from concourse import bass, mybir, tile

    f32 = mybir.dt.float32
    bf16 = mybir.dt.bfloat16
    f8 = mybir.dt.float8e4
    u16 = mybir.dt.uint16
    i32 = mybir.dt.int32
    Tanh = mybir.ActivationFunctionType.Tanh
    Exp = mybir.ActivationFunctionType.Exp
    Alu = mybir.AluOpType
    Ax = mybir.AxisListType
    DR = mybir.MatmulPerfMode.DoubleRow

    nc = bass.Bass()

    enc_d = nc.declare_dram_parameter("encoder_outputs", [BL, S, E], f32, isOutput=False)
    # one bounce tensor per batch: no shared-tensor dep tracking.
    # batch 0 additionally gets per-half tensors so its first transpose
    # only RAW-depends on half the casts (startup fill).
    enc8_d = [nc.dram_tensor(f"enc8_{b}", [S, E], f8) for b in range(BL)]
    enc8_b0h = [nc.dram_tensor(f"enc8_0h{h}", [NB, E], f8) for h in range(SB)]
    mask_d = nc.declare_dram_parameter("mask", [BL, S], i32, isOutput=False)
    wh_d = nc.declare_dram_parameter("wh_pack", [128, KT, K], bf16, isOutput=False)
    w8_d = nc.declare_dram_parameter("w8_pack", [128, ET2, 2, K], f8, isOutput=False)
    hT_d = nc.declare_dram_parameter("hT_pack", [128, KT * BL], bf16, isOutput=False)
    b_d = nc.declare_dram_parameter("b_pack", [1, K], bf16, isOutput=False)
    v_d = nc.declare_dram_parameter("v_pack", [128, KT], bf16, isOutput=False)
    out_d = nc.declare_dram_parameter("out", [BL, S], f32, isOutput=True)

    with tile.TileContext(nc) as tc, ExitStack() as ctx:
        const = ctx.enter_context(tc.tile_pool(name="const", bufs=1))
        tp_pool = ctx.enter_context(tc.tile_pool(name="encT", bufs=3))
        tanh_pool = ctx.enter_context(tc.tile_pool(name="tanh", bufs=6))
        pre_ps = ctx.enter_context(tc.tile_pool(name="pre_ps", bufs=3, space="PSUM"))
        sc_ps = ctx.enter_context(tc.tile_pool(name="sc_ps", bufs=1, space="PSUM"))
        fin = ctx.enter_context(tc.tile_pool(name="fin", bufs=1))

        # ---- weights (host-packed) ----
        # w8 gates the first main matmul: first on the sync ring, in two
        # DMAs so they overlap (per-DMA rate is ~143 GB/s)
        w8 = const.tile([128, ET2, 2, K], f8)
        nc.sync.dma_start(w8[:, :2], w8_d[:, :2])
        nc.sync.dma_start(w8[:, 2:], w8_d[:, 2:])
        hT_bf = const.tile([128, KT * BL], bf16)
        nc.sync.dma_start(hT_bf[:], hT_d[:])
        b_attn_bf = const.tile([1, K], bf16)
        nc.sync.dma_start(b_attn_bf[:], b_d[:])
        v_bf = const.tile([128, KT], bf16)
        nc.sync.dma_start(v_bf[:], v_d[:])
        # wh per-kt chunks on the scalar ring (repacked kt-major on host)
        wh_bf = const.tile([128, KT, KT, 128], bf16)  # [p, kt, dt, c]

        ones_bf = const.tile([1, BL], bf16)
        nc.vector.memset(ones_bf[:], 1.0)
        scratch = const.tile([1, K], bf16)

        def dummy_dma():
            # 2KB no-op DMA: pads the semaphore-pool rotation in the
            # PROLOGUE ONLY, so the first transposes' semaphores are reused
            # by instantly-completing dummies instead of later cast chunks
            # (readers get bumped to a semaphore's next user: the first MM
            # was waiting a b1 cast chunk at ~29us, b1's MMs waited tp(b2))
            nc.sync.dma_start(scratch[:], b_d[:])

        def stage_cast(b):
            """f32 -> fp8 DRAM->DRAM SWDGE cast of one batch, in 8 chunks
            round-robined over the 8 SWDGE queues (the ~300 GB/s read side
            is the limiter; the 1MB fp8 write hides inside the 4MB read)."""
            for st in range(ST):
                nc.gpsimd.dma_start(
                    enc8_d[b][st * 128:(st + 1) * 128, :],
                    enc_d[b, st * 128:(st + 1) * 128, :])

        def stage_tp(b):
            """DRAM->SBUF u16-pair xbar transpose of the fp8 bounce:
            encT8 u16[p, et, s] = fp8 pair
            (enc[b, s, et*256+2p], enc[b, s, et*256+2p+1]).
            Every transpose is a full DMA-system barrier (Tile fences it
            against ALL in-flight DMAs, both directions), so the schedule
            strictly alternates cast(batch) / tp(batch) — ONE fence cycle
            per batch; splitting into halves costs a second ~5us hop."""
            eh = tp_pool.tile([128, ET2, S], u16, tag="encT",
                              name=f"encT_{b}")
            # ONE full-size tp per batch: half-splitting was tried twice and
            # loses ~2-5us/batch — the halves interleave with cast groups
            # into two fence cycles per batch
            nc.sync.dma_start(eh[:], enc8_d[b][:].bitcast(u16),
                              transpose=True)
            return eh

        def stage_cast_b0h(h):
            for st in range(ST // SB):
                nc.gpsimd.dma_start(
                    enc8_b0h[h][st * 128:(st + 1) * 128, :],
                    enc_d[0, h * NB + st * 128:h * NB + (st + 1) * 128, :])

        def stage_tp_b0h(h):
            ehh = tp_pool.tile([128, ET2, NB], u16, tag="encTh",
                               name=f"encT0h{h}", bufs=2)
            nc.sync.dma_start(ehh[:], enc8_b0h[h][:].bitcast(u16),
                              transpose=True)
            return ehh

        def rhs_view(eh, f8dt):
            return eh[:].bitcast(f8dt).rearrange("p et (s j) -> p et j s", j=2)

        hpb = const.tile([128, KT * BL], f32)  # col = kt*BL + b

        def emit_hp_kt(kt):
            # h_proj[k, b] = sum_d Wh[d, k]*hidden[b, d] + b_attn[k],
            # one k-tile at a time, interleaved into batch 0's MM stream
            hp_ps = pre_ps.tile([128, NB], f32, tag="pre", name=f"hp_ps{kt}",
                                bufs=7)
            for dt in range(KT):
                nc.tensor.matmul(
                    hp_ps[:, :BL],
                    wh_bf[:, kt, dt, :],
                    hT_bf[:, dt * BL:(dt + 1) * BL],
                    start=(dt == 0),
                    stop=False,
                )
            nc.tensor.matmul(
                hp_ps[:, :BL],
                b_attn_bf[:, kt * 128:(kt + 1) * 128],
                ones_bf[:],
                start=False,
                stop=True,
            )
            # ACT, not DVE: the DVE is busy with next-batch casts at b0
            # start and the first tanh would block behind them (gpsimd
            # cannot read PSUM)
            nc.scalar.copy(hpb[:, kt * BL:(kt + 1) * BL], hp_ps[:, :BL])

        # scores accumulate on PSUM rows 32q (col-group q = s-quarter q);
        # staged in flat4 rows 32q on SBUF, gathered to [b, S] chunks
        flat4 = fin.tile([97, BL * 256], f32)

        # softmax state.  scores is pre-filled with (mask-1)*1e10 at mask
        # prep; the gathers then ACCUMULATE flat4 into it (SWDGE add), so
        # the masked add costs nothing on the tail chain.
        mask_i = fin.tile([BL, S], i32)
        mask_f = fin.tile([BL, S], f32)
        scores = fin.tile([BL, S], f32)
        negmax = fin.tile([BL, 1], f32)
        expv = fin.tile([BL, S], f32)
        rowsum = fin.tile([BL, 1], f32)
        recip = fin.tile([BL, 1], f32)
        outf = fin.tile([BL, S], f32)

        def emit_mask_prep():
            nc.scalar.dma_start(mask_i[:], mask_d[:])
            nc.vector.tensor_copy(mask_f[:], mask_i[:])
            nc.vector.tensor_scalar(
                scores[:], mask_f[:], -NEG, NEG, Alu.mult, Alu.add)

        def emit_gathers(b0, b1):
            # gather flat4 quarter-rows into [b, s] layout; SWDGE spreads
            # the 4 tiny SBUF->SBUF copies over 4 queues (parallel) — only
            # emitted after the last transpose, so no xbar fence stalls
            for q in range(4):
                nc.gpsimd.dma_start(
                    scores[b0:b1, q * 256:(q + 1) * 256],
                    flat4[32 * q:32 * q + 1, b0 * 256:b1 * 256],
                    accum_op=Alu.add)

        def emit_softmax_chunk(b0, b1, gather=True):
            if gather:
                emit_gathers(b0, b1)
            # compute ops always span [0, b1): DVE/ACT partition windows
            # must start at 0 (quadrant alignment); recomputing the already
            # finished low batches is idempotent and partition-parallel
            nc.vector.tensor_reduce(
                negmax[:b1], scores[:b1, :], Ax.X, Alu.max, negate=True)
            nc.scalar.activation(
                expv[:b1, :], scores[:b1, :], Exp, bias=negmax[:b1],
                accum_out=rowsum[:b1])
            nc.vector.reciprocal(recip[:b1], rowsum[:b1])
            nc.vector.tensor_scalar_mul(
                outf[:b1, :], expv[:b1, :], recip[:b1])
            nc.scalar.dma_start(out_d[b0:b1, :], outf[b0:b1, :])

        # ---- prologue ----
        # wh chunks on the scalar ring (free until the gathers at the end)
        for ck in range(KT):
            # wh_d dims are [p, kt, (dt c)] after the host kt-major repack
            nc.scalar.dma_start(wh_bf[:, ck], wh_d[:, ck])
        encTs = {}

        # batch 0 runs at HALF granularity so the PE starts ~23us earlier:
        # [cast b0h0, tp b0h0] -> sb0 MM pass (emitted HERE so its
        # conservative last-transpose wait is tp(b0h0), not a later tp) ->
        # [tp b0h1, cast b1] -> sb1 pass -> [tp b1, cast b2] -> main loop
        stage_cast_b0h(0)
        encT0h = [stage_tp_b0h(0)]
        stage_cast_b0h(1)
        for _ in range(5):
            dummy_dma()

        # scores PSUM: ONE bank, halves alternated by batch parity
        scband = sc_ps.tile([128, 2, 256], f32, tag="sc", name="scband")

        def scq(b, q):
            return scband[32 * q:32 * q + 1, b % 2, :]

        carry = []  # [(b_prev, kt, th_tile, col_off, qs)] not yet emitted

        def emit_vdots(b, kt, th, col_off, qs, stop):
            for q in qs:
                nc.tensor.matmul(
                    scq(b, q),
                    v_bf[:, kt:kt + 1],
                    th[:, col_off + (q - qs[0]) * 256:
                       col_off + (q - qs[0] + 1) * 256],
                    start=(kt == 0), stop=stop,
                    tile_position=(0, 32 * q))
            if stop:
                # all MMs above before any copy: interleaving creates false
                # column-range WARs that bubble the PE ~850ns per quarter
                for q in qs:
                    nc.vector.tensor_copy(
                        flat4[32 * q:32 * q + 1, b * 256:(b + 1) * 256],
                        scq(b, q))

        def emit_carry(budget):
            while carry and budget > 0:
                b_p, kt, th, col_off, qs = carry.pop(0)
                emit_vdots(b_p, kt, th, col_off, qs, stop=(kt == KT - 1))
                budget -= 1

        def emit_b0_pass(sb, eh):
            rh = rhs_view(eh, f8)
            for kt in range(KT):
                pre = pre_ps.tile([128, NB], f32, tag="pre",
                                  name="preh", bufs=7)
                for et in range(ET2):
                    nc.tensor.matmul(
                        pre[:],
                        w8[:, et, :, kt * 128:(kt + 1) * 128],
                        rh[:, et, :, :],
                        start=(et == 0),
                        stop=(et == ET2 - 1),
                        perf_mode=DR,
                    )
                if sb == 0:
                    emit_hp_kt(kt)
                th = tanh_pool.tile([128, NB], bf16, tag="thh",
                                    name="thh", bufs=5)
                nc.scalar.activation(
                    th[:], pre[:], Tanh,
                    bias=hpb[:, kt * BL:kt * BL + 1],
                    scale=1.0 / WSCALE,
                )
                if kt >= 1:
                    emit_vdots(0, kt - 1, b0_ths[(sb, kt - 1)], 0,
                               (2 * sb, 2 * sb + 1), stop=False)
                b0_ths[(sb, kt)] = th
            carry.append((0, KT - 1, b0_ths[(sb, KT - 1)], 0,
                          (2 * sb, 2 * sb + 1)))

        b0_ths = {}
        emit_b0_pass(0, encT0h[0])
        encT0h.append(stage_tp_b0h(1))
        stage_cast(1)
        dummy_dma()
        emit_b0_pass(1, encT0h[1])
        encTs[1] = stage_tp(1)
        stage_cast(2)
        dummy_dma()

        # ---- main loop over local batches (software-pipelined) ----
        for b in range(1, BL):
            if b == 1:
                emit_mask_prep()

            rhv = rhs_view(encTs.pop(b), f8)
            ths = {}
            for kt in range(KT):
                pres = [pre_ps.tile([128, NB], f32, tag="pre",
                                    name=f"pre{sb}", bufs=7)
                        for sb in range(SB)]
                for et in range(ET2):  # one LDWEIGHTS serves both sb
                    for sb in range(SB):
                        nc.tensor.matmul(
                            pres[sb][:],
                            w8[:, et, :, kt * 128:(kt + 1) * 128],
                            rhv[:, et, :, sb * NB:(sb + 1) * NB],
                            start=(et == 0),
                            stop=(et == ET2 - 1),
                            perf_mode=DR,
                        )
                if b == 0:
                    # hidden@Wh one k-tile at a time, off the startup
                    # critical path (wh streams in per-kt chunks)
                    emit_hp_kt(kt)
                elif kt < VLAG:
                    emit_carry(1)
                th = tanh_pool.tile([128, SB * NB], bf16, tag="tanh",
                                    bufs=6)
                for sb in range(SB):
                    nc.scalar.activation(
                        th[:, sb * NB:(sb + 1) * NB], pres[sb][:], Tanh,
                        bias=hpb[:, kt * BL + b:kt * BL + b + 1],
                        scale=1.0 / WSCALE,
                    )
                ths[kt] = th
                if kt >= VLAG:
                    emit_vdots(b, kt - VLAG, ths[kt - VLAG], 0,
                               (0, 1, 2, 3), stop=False)

            for kt in range(KT - VLAG, KT):
                carry.append((b, kt, ths[kt], 0, (0, 1, 2, 3)))
            # stage the next batches AFTER this batch's MMs (readers of a
            # transpose-written tile conservatively wait on the last
            # transpose emitted before them); tp(b+1) first: its cast
            # finished last iteration, so encT(b+1) lands early with slack
            if b + 1 < BL:
                encTs[b + 1] = stage_tp(b + 1)
            if b + 2 < BL:
                stage_cast(b + 2)
            if b == 7:
                emit_softmax_chunk(0, 6)

        emit_gathers(6, 7)  # b6's flat4 is final; runs during b7's tail MMs
        emit_carry(len(carry))
        emit_gathers(7, 8)
        emit_softmax_chunk(6, 8, gather=False)

    if strip:
        _split_multi_waits(nc, mybir)
    return nc


def _split_multi_waits(nc, mybir):
    """Move extra semaphore waits onto standalone NoOps on the same engine.

    This walrus build encodes at most one sync-wait command per instruction,
    but Tile emits instructions with several (cross-engine RAW + WAR + DMA
    queue ordering). A NoOp carrying one wait, placed immediately before the
    instruction in the same engine's stream, is semantically identical: the
    engine's sequencer blocks on the NoOp's wait before dispatching the real
    instruction.
    """
    n = 0
    for fn in nc.m.functions:
        for blk in fn.blocks:
            insts = blk.instructions
            new = []
            changed = False
            for inst in insts:
                si = inst.sync_info
                if si is not None and si.on_wait and len(si.on_wait) > 1:
                    for w in list(si.on_wait)[:-1]:
                        n += 1
                        new.append(mybir.InstNoOp(
                            name=f"{inst.name}-sw{n}",
                            engine=inst.engine,
                            text_hint="split_wait",
                            bass_nofuse=True,
                            sync_info=mybir.SyncInfo(
                                on_wait=[w], on_update=[]),
                        ))
                    inst.sync_info = mybir.SyncInfo(
                        on_wait=[list(si.on_wait)[-1]],
                        on_update=list(si.on_update or []))
                    changed = True
                new.append(inst)
            if changed:
                blk.instructions = new


def get_nc(strip=True):
    key = ("nc", strip)
    if key not in _CACHE:
        _CACHE[key] = _build_bass(strip)
    return _CACHE[key]


def make_in_maps(hidden, encoder_outputs, mask, W_attn, b_attn, v):
    import ml_dtypes

    bf16 = ml_dtypes.bfloat16
    f8 = ml_dtypes.float8_e4m3

    W_attn = np.asarray(W_attn, dtype=np.float32)
    Wh, We = W_attn[:K], W_attn[K:]
    # wh_pack[p, kt, dt, c] = Wh[dt*128 + p, kt*128 + c]  (kt-major chunks)
    wh_pack = np.ascontiguousarray(
        Wh.reshape(KT, 128, KT, 128).transpose(1, 2, 0, 3).astype(bf16))
    # w8_pack[p, et, j, k] = 64 * We[et*256 + 2p + j, k]
    w8_pack = np.ascontiguousarray(
        (We * WSCALE).reshape(ET2, 128, 2, K).transpose(1, 0, 2, 3).astype(f8))
    b_pack = np.ascontiguousarray(
        np.asarray(b_attn, dtype=np.float32).reshape(1, K).astype(bf16))
    # v_pack[p, kt] = v[kt*128 + p]
    v_pack = np.ascontiguousarray(
        np.asarray(v, dtype=np.float32).reshape(KT, 128).T.astype(bf16))
    hidden = np.asarray(hidden, dtype=np.float32)

    in_maps = []
    for c in range(NCORES):
        sl = slice(c * BL, (c + 1) * BL)
        # hT_pack[p, dt*BL + b] = hidden[b, dt*128 + p]
        hT_pack = np.ascontiguousarray(
            hidden[sl].T.reshape(KT, 128, BL).transpose(1, 0, 2)
            .reshape(128, KT * BL).astype(bf16))
        in_maps.append({
            "encoder_outputs": np.ascontiguousarray(encoder_outputs[sl]),
            "mask": np.ascontiguousarray(np.asarray(mask[sl], dtype=np.int32)),
            "wh_pack": wh_pack,
            "w8_pack": w8_pack,
            "hT_pack": hT_pack,
            "b_pack": b_pack,
            "v_pack": v_pack,
        })
    return in_maps


def kernel(hidden, encoder_outputs, mask, W_attn, b_attn, v):
    from concourse.bass_utils import run_bass_kernel_spmd

    nc = get_nc()
    in_maps = make_in_maps(hidden, encoder_outputs, mask, W_attn, b_attn, v)
    res = run_bass_kernel_spmd(nc, in_maps, core_ids=list(range(NCORES)))
    return np.concatenate(
        [np.asarray(res.results[c]["out"], dtype=np.float32) for c in range(NCORES)],
        axis=0,
    )


# revision 59
# speedup vs baseline: 1.0700x; 1.0192x over previous
"""Bahdanau-attention scores kernel for Trainium2 (8 NeuronCores, SPMD).

Computation (per batch row b):
    pre[s, k] = hidden[b] @ Wh + enc[b, s] @ We + b_attn       (S=1024, E=K=1024)
    scores[s] = tanh(pre[s, :]) @ v
    out[b]    = softmax(where(mask[b]==0, -1e10, scores))      over s

Sharding: data-parallel over batch B=64 -> 8 batches per core; weights
replicated. No collectives.

Per-core structure (fp8 DoubleRow main matmul, bf16 elsewhere):
  - enc pipeline per batch: DRAM->DRAM SWDGE cast f32->fp8e4 (8 chunks
    over the 8 SWDGE queues, ~13us: ~300 GB/s read-side; the 1MB fp8
    write hides inside the 4MB read), then ONE full-batch DRAM->SBUF xbar
    transpose of the fp8 pairs as u16 (issue ~4us + drain ~5us):
    encT8[p, et, s] u16 = (enc[s, et*256+2p], enc[s, et*256+2p+1]) --
    exactly the DoubleRow rhs pairing.
  - THE governing constraint: Tile fences every xbar transpose against
    ALL in-flight DMAs on every queue (SWDGE + both HWDGE rings, both
    directions).  All DMA time is therefore strictly additive around each
    transpose; the steady-state floor is cast(13) + tp(9) ~= 22us/batch,
    above the PE's ~17us.  Consequences baked into the schedule:
      * ONE transpose per batch (half-splits pay a second ~5us fence hop
        and interleave with cast groups -- measured worse twice);
      * per-iteration emission: [batch-b MM loop] -> [tp(b+1)] ->
        [cast(b+2)], so exactly one tp-block and one cast-block alternate
        per fence cycle; tp first, because its dependency (cast(b+1))
        finished last iteration, so encT(b+1) lands with slack and run-to-
        run jitter stops re-throttling HAM at batch starts;
      * MMs are emitted BEFORE the next tp: readers of a transpose-written
        tile conservatively wait on the LAST transpose emitted before
        them (shared HWDGE semaphore pool), so a tp emitted before the
        MMs that consume the PREVIOUS tp adds a spurious ~10us stall;
      * alternatives measured and rejected: HWDGE loads + DVE cast +
        bounce store (SBUF->DRAM stores cap at ~87 GB/s on every path,
        and HWDGE rings allow only ~3 in-flight DMAs with ~143 GB/s per
        DMA); strided truncated-bf16 loads (DMA needs a contiguous last
        dim); DVE 32x32 stream-transpose (32-lane limited, ~12us/batch).
  - one DRAM bounce tensor per batch so coarse DRAM-range tracking never
    serializes different batches' casts/transposes.
  - main MM: pre[k, s] = sum_et lhsT(w8) @ rhs(encT8), DoubleRow, one
    LDWEIGHTS per (et, kt) serving both s-halves; ~216ns/MM warm.
  - ScalarE: tanh(psum/64 + (hidden@Wh + b_attn)[k]) -> SBUF bf16
  - hidden@Wh (hp) is interleaved per-kt into batch 0's loop (wh loaded as
    per-kt chunks on the scalar ring) so it doesn't sit at the PE FIFO
    head blocking the first main MMs behind a 2MB weight load.
  - v-dot: 4 col-tiled PE matmuls (tile_position=(0,32q)), lag FOUR
    k-tiles behind the main MMs (ACT falls ~1.5 groups behind the PE by
    batch end; lag 2 stalled the PE 0.5-1.5us per batch).  Last 4 k-tiles
    carried into the next batch, one k-tile per MM group, all 4 quarter
    MMs before the 4 flat4 copies (interleaving MMs and copies created
    false column-range WARs = 3x850ns PE bubbles).  tanh pool is 6 deep so
    the carried v-dots' reads never make the next batch's tanh wait.
  - softmax in chunks: batches 0-5 during iter 7, batch 6 right after its
    carry, 7 alone on the tail.  scores is pre-filled with (mask-1)*1e10;
    gathers flat4->scores are SWDGE accumulate-adds spread over 4 queues
    (on the scalar/ACT ring they blocked the tanh stream: -29us).  The
    final chunk recomputes rows 0..7 (DVE/ACT partition windows must
    start at partition 0; the recompute is idempotent and lane-parallel).

Sync note: this walrus build encodes at most ONE semaphore wait per
instruction; _split_multi_waits() rewrites Tile's multi-wait instructions
into NoOp(wait) chains on the same engine.
"""

import sys

if "/opt/trn_rl_repo" not in sys.path:
    sys.path.insert(0, "/opt/trn_rl_repo")

from contextlib import ExitStack

import numpy as np

B, S, E, K = 64, 1024, 1024, 1024  # E = 2*ENC_HID, K = DEC_HID
NCORES = 8
BL = B // NCORES  # batches per core
NEG = -1e10
WSCALE = 64.0     # We quantization scale into E4M3 range

ET2 = E // 256  # 4 DoubleRow e-tiles (256-deep contraction each)
KT = K // 128   # 8 k-tiles
ST = S // 128   # 8 s-tiles
NB = 512        # matmul free-dim block (one s-half)
SB = S // NB    # 2 s-halves
VLAG = 4        # v-dot lag in k-tiles behind the main MMs

_CACHE = {}


def _build_bass(strip=True):
    from concourse import bass, mybir, tile

    f32 = mybir.dt.float32
    bf16 = mybir.dt.bfloat16
    f8 = mybir.dt.float8e4
    u16 = mybir.dt.uint16
    i32 = mybir.dt.int32
    Tanh = mybir.ActivationFunctionType.Tanh
    Exp = mybir.ActivationFunctionType.Exp
    Alu = mybir.AluOpType
    Ax = mybir.AxisListType
    DR = mybir.MatmulPerfMode.DoubleRow

    nc = bass.Bass()

    enc_d = nc.declare_dram_parameter("encoder_outputs", [BL, S, E], f32, isOutput=False)
    # one bounce tensor per batch: no shared-tensor dep tracking.
    # batch 0 additionally gets per-half tensors so its first transpose
    # only RAW-depends on half the casts (startup fill).
    enc8_d = [nc.dram_tensor(f"enc8_{b}", [S, E], f8) for b in range(BL)]
    enc8_b0h = [nc.dram_tensor(f"enc8_0h{h}", [NB, E], f8) for h in range(SB)]
    mask_d = nc.declare_dram_parameter("mask", [BL, S], i32, isOutput=False)
    wh_d = nc.declare_dram_parameter("wh_pack", [128, KT, K], bf16, isOutput=False)
    w8_d = nc.declare_dram_parameter("w8_pack", [128, ET2, 2, K], f8, isOutput=False)
    hT_d = nc.declare_dram_parameter("hT_pack", [128, KT * BL], bf16, isOutput=False)
    b_d = nc.declare_dram_parameter("b_pack", [1, K], bf16, isOutput=False)
    v_d = nc.declare_dram_parameter("v_pack", [128, KT], bf16, isOutput=False)
    out_d = nc.declare_dram_parameter("out", [BL, S], f32, isOutput=True)

    with tile.TileContext(nc) as tc, ExitStack() as ctx:
        const = ctx.enter_context(tc.tile_pool(name="const", bufs=1))
        tp_pool = ctx.enter_context(tc.tile_pool(name="encT", bufs=3))
        tanh_pool = ctx.enter_context(tc.tile_pool(name="tanh", bufs=6))
        pre_ps = ctx.enter_context(tc.tile_pool(name="pre_ps", bufs=3, space="PSUM"))
        sc_ps = ctx.enter_context(tc.tile_pool(name="sc_ps", bufs=1, space="PSUM"))
        fin = ctx.enter_context(tc.tile_pool(name="fin", bufs=1))

        # ---- weights (host-packed) ----
        # w8 gates the first main matmul: first on the sync ring, in two
        # DMAs so they overlap (per-DMA rate is ~143 GB/s)
        w8 = const.tile([128, ET2, 2, K], f8)
        nc.sync.dma_start(w8[:, :2], w8_d[:, :2])
        nc.sync.dma_start(w8[:, 2:], w8_d[:, 2:])
        hT_bf = const.tile([128, KT * BL], bf16)
        nc.sync.dma_start(hT_bf[:], hT_d[:])
        b_attn_bf = const.tile([1, K], bf16)
        nc.sync.dma_start(b_attn_bf[:], b_d[:])
        v_bf = const.tile([128, KT], bf16)
        nc.sync.dma_start(v_bf[:], v_d[:])
        # wh per-kt chunks on the scalar ring (repacked kt-major on host)
        wh_bf = const.tile([128, KT, KT, 128], bf16)  # [p, kt, dt, c]

        ones_bf = const.tile([1, BL], bf16)
        nc.vector.memset(ones_bf[:], 1.0)

        def stage_cast(b):
            """f32 -> fp8 DRAM->DRAM SWDGE cast of one batch, in 8 chunks
            round-robined over the 8 SWDGE queues (the ~300 GB/s read side
            is the limiter; the 1MB fp8 write hides inside the 4MB read)."""
            for st in range(ST):
                nc.gpsimd.dma_start(
                    enc8_d[b][st * 128:(st + 1) * 128, :],
                    enc_d[b, st * 128:(st + 1) * 128, :])

        def stage_tp(b):
            """DRAM->SBUF u16-pair xbar transpose of the fp8 bounce:
            encT8 u16[p, et, s] = fp8 pair
            (enc[b, s, et*256+2p], enc[b, s, et*256+2p+1]).
            Every transpose is a full DMA-system barrier (Tile fences it
            against ALL in-flight DMAs, both directions), so the schedule
            strictly alternates cast(batch) / tp(batch) — ONE fence cycle
            per batch; splitting into halves costs a second ~5us hop."""
            eh = tp_pool.tile([128, ET2, S], u16, tag="encT",
                              name=f"encT_{b}")
            # ONE full-size tp per batch: half-splitting was tried twice and
            # loses ~2-5us/batch — the halves interleave with cast groups
            # into two fence cycles per batch
            nc.sync.dma_start(eh[:], enc8_d[b][:].bitcast(u16),
                              transpose=True)
            return eh

        def stage_cast_b0h(h):
            for st in range(ST // SB):
                nc.gpsimd.dma_start(
                    enc8_b0h[h][st * 128:(st + 1) * 128, :],
                    enc_d[0, h * NB + st * 128:h * NB + (st + 1) * 128, :])

        def stage_tp_b0h(h):
            ehh = tp_pool.tile([128, ET2, NB], u16, tag="encTh",
                               name=f"encT0h{h}", bufs=2)
            nc.sync.dma_start(ehh[:], enc8_b0h[h][:].bitcast(u16),
                              transpose=True)
            return ehh

        def rhs_view(eh, f8dt):
            return eh[:].bitcast(f8dt).rearrange("p et (s j) -> p et j s", j=2)

        hpb = const.tile([128, KT * BL], f32)  # col = kt*BL + b

        def emit_hp_kt(kt):
            # h_proj[k, b] = sum_d Wh[d, k]*hidden[b, d] + b_attn[k],
            # one k-tile at a time, interleaved into batch 0's MM stream
            hp_ps = pre_ps.tile([128, NB], f32, tag="pre", name=f"hp_ps{kt}",
                                bufs=7)
            for dt in range(KT):
                nc.tensor.matmul(
                    hp_ps[:, :BL],
                    wh_bf[:, kt, dt, :],
                    hT_bf[:, dt * BL:(dt + 1) * BL],
                    start=(dt == 0),
                    stop=False,
                )
            nc.tensor.matmul(
                hp_ps[:, :BL],
                b_attn_bf[:, kt * 128:(kt + 1) * 128],
                ones_bf[:],
                start=False,
                stop=True,
            )
            # ACT, not DVE: the DVE is busy with next-batch casts at b0
            # start and the first tanh would block behind them (gpsimd
            # cannot read PSUM)
            nc.scalar.copy(hpb[:, kt * BL:(kt + 1) * BL], hp_ps[:, :BL])

        # scores accumulate on PSUM rows 32q (col-group q = s-quarter q);
        # staged in flat4 rows 32q on SBUF, gathered to [b, S] chunks
        flat4 = fin.tile([128, BL * 256], f32)

        # softmax state.  scores is pre-filled with (mask-1)*1e10 at mask
        # prep; the gathers then ACCUMULATE flat4 into it (SWDGE add), so
        # the masked add costs nothing on the tail chain.
        mask_i = fin.tile([BL, S], i32)
        mask_f = fin.tile([BL, S], f32)
        scores = fin.tile([BL, S], f32)
        negmax = fin.tile([BL, 1], f32)
        expv = fin.tile([BL, S], f32)
        rowsum = fin.tile([BL, 1], f32)
        recip = fin.tile([BL, 1], f32)
        outf = fin.tile([BL, S], f32)

        def emit_mask_prep():
            nc.scalar.dma_start(mask_i[:], mask_d[:])
            nc.vector.tensor_copy(mask_f[:], mask_i[:])
            nc.vector.tensor_scalar(
                scores[:], mask_f[:], -NEG, NEG, Alu.mult, Alu.add)

        def emit_gathers(b0, b1):
            # gather flat4 quarter-rows into [b, s] layout (SWDGE accum;
            # only emitted after the last transpose -> no xbar fence stalls)
            if b1 - b0 == 1:
                # single-batch: ONE partition-strided DMA instead of four
                # issue-staggered single-partition reads (~-2.5us on the
                # b7 tail chain)
                nc.gpsimd.dma_start(
                    scores[b0:b1, :],
                    flat4[:].rearrange("(q r) c -> q r c", r=32)[
                        :, 0, b0 * 256:b1 * 256],
                    accum_op=Alu.add)
                return
            for q in range(4):
                nc.gpsimd.dma_start(
                    scores[b0:b1, q * 256:(q + 1) * 256],
                    flat4[32 * q:32 * q + 1, b0 * 256:b1 * 256],
                    accum_op=Alu.add)

        def emit_softmax_chunk(b0, b1, gather=True):
            if gather:
                emit_gathers(b0, b1)
            # compute ops always span [0, b1): DVE/ACT partition windows
            # must start at 0 (quadrant alignment); recomputing the already
            # finished low batches is idempotent and partition-parallel
            nc.vector.tensor_reduce(
                negmax[:b1], scores[:b1, :], Ax.X, Alu.max, negate=True)
            nc.scalar.activation(
                expv[:b1, :], scores[:b1, :], Exp, bias=negmax[:b1],
                accum_out=rowsum[:b1])
            nc.vector.reciprocal(recip[:b1], rowsum[:b1])
            nc.vector.tensor_scalar_mul(
                outf[:b1, :], expv[:b1, :], recip[:b1])
            nc.scalar.dma_start(out_d[b0:b1, :], outf[b0:b1, :])

        # ---- prologue ----
        # wh chunks on the scalar ring (free until the gathers at the end)
        for ck in range(KT):
            # wh_d dims are [p, kt, (dt c)] after the host kt-major repack
            nc.scalar.dma_start(wh_bf[:, ck], wh_d[:, ck])
        encTs = {}

        # batch 0 runs at HALF granularity so the PE starts ~23us earlier:
        # [cast b0h0, tp b0h0] -> sb0 MM pass (emitted HERE so its
        # conservative last-transpose wait is tp(b0h0), not a later tp) ->
        # [tp b0h1, cast b1] -> sb1 pass -> [tp b1, cast b2] -> main loop
        stage_cast_b0h(0)
        encT0h = [stage_tp_b0h(0)]
        stage_cast_b0h(1)

        # scores PSUM: ONE bank, halves alternated by batch parity
        scband = sc_ps.tile([128, 2, 256], f32, tag="sc", name="scband")

        def scq(b, q):
            return scband[32 * q:32 * q + 1, b % 2, :]

        carry = []  # [(b_prev, kt, th_tile, col_off, qs)] not yet emitted

        def emit_vdots(b, kt, th, col_off, qs, stop):
            for q in qs:
                nc.tensor.matmul(
                    scq(b, q),
                    v_bf[:, kt:kt + 1],
                    th[:, col_off + (q - qs[0]) * 256:
                       col_off + (q - qs[0] + 1) * 256],
                    start=(kt == 0), stop=stop,
                    tile_position=(0, 32 * q))
            if stop:
                # all MMs above before any copy: interleaving creates false
                # column-range WARs that bubble the PE ~850ns per quarter
                for q in qs:
                    nc.vector.tensor_copy(
                        flat4[32 * q:32 * q + 1, b * 256:(b + 1) * 256],
                        scq(b, q))

        def emit_carry(budget):
            while carry and budget > 0:
                b_p, kt, th, col_off, qs = carry.pop(0)
                emit_vdots(b_p, kt, th, col_off, qs, stop=(kt == KT - 1))
                budget -= 1

        def emit_b0_pass(sb, eh):
            rh = rhs_view(eh, f8)
            for kt in range(KT):
                pre = pre_ps.tile([128, NB], f32, tag="pre",
                                  name="preh", bufs=7)
                for et in range(ET2):
                    nc.tensor.matmul(
                        pre[:],
                        w8[:, et, :, kt * 128:(kt + 1) * 128],
                        rh[:, et, :, :],
                        start=(et == 0),
                        stop=(et == ET2 - 1),
                        perf_mode=DR,
                    )
                if sb == 0:
                    emit_hp_kt(kt)
                th = tanh_pool.tile([128, NB], bf16, tag="thh",
                                    name="thh", bufs=5)
                nc.scalar.activation(
                    th[:], pre[:], Tanh,
                    bias=hpb[:, kt * BL:kt * BL + 1],
                    scale=1.0 / WSCALE,
                )
                if kt >= 1:
                    emit_vdots(0, kt - 1, b0_ths[(sb, kt - 1)], 0,
                               (2 * sb, 2 * sb + 1), stop=False)
                b0_ths[(sb, kt)] = th
            carry.append((0, KT - 1, b0_ths[(sb, KT - 1)], 0,
                          (2 * sb, 2 * sb + 1)))

        b0_ths = {}
        emit_b0_pass(0, encT0h[0])
        encT0h.append(stage_tp_b0h(1))
        stage_cast(1)
        emit_b0_pass(1, encT0h[1])
        encTs[1] = stage_tp(1)
        stage_cast(2)

        # ---- main loop over local batches (software-pipelined) ----
        for b in range(1, BL):
            if b == 1:
                emit_mask_prep()

            rhv = rhs_view(encTs.pop(b), f8)
            ths = {}
            for kt in range(KT):
                pres = [pre_ps.tile([128, NB], f32, tag="pre",
                                    name=f"pre{sb}", bufs=7)
                        for sb in range(SB)]
                for et in range(ET2):  # one LDWEIGHTS serves both sb
                    for sb in range(SB):
                        nc.tensor.matmul(
                            pres[sb][:],
                            w8[:, et, :, kt * 128:(kt + 1) * 128],
                            rhv[:, et, :, sb * NB:(sb + 1) * NB],
                            start=(et == 0),
                            stop=(et == ET2 - 1),
                            perf_mode=DR,
                        )
                if b == 0:
                    # hidden@Wh one k-tile at a time, off the startup
                    # critical path (wh streams in per-kt chunks)
                    emit_hp_kt(kt)
                elif kt < VLAG:
                    emit_carry(1)
                th = tanh_pool.tile([128, SB * NB], bf16, tag="tanh",
                                    bufs=6)
                for sb in range(SB):
                    nc.scalar.activation(
                        th[:, sb * NB:(sb + 1) * NB], pres[sb][:], Tanh,
                        bias=hpb[:, kt * BL + b:kt * BL + b + 1],
                        scale=1.0 / WSCALE,
                    )
                ths[kt] = th
                if kt >= VLAG:
                    emit_vdots(b, kt - VLAG, ths[kt - VLAG], 0,
                               (0, 1, 2, 3), stop=False)

            for kt in range(KT - VLAG, KT):
                carry.append((b, kt, ths[kt], 0, (0, 1, 2, 3)))
            # stage the next batches AFTER this batch's MMs (readers of a
            # transpose-written tile conservatively wait on the last
            # transpose emitted before them); tp(b+1) first: its cast
            # finished last iteration, so encT(b+1) lands early with slack
            if b + 1 < BL:
                encTs[b + 1] = stage_tp(b + 1)
            if b + 2 < BL:
                stage_cast(b + 2)
            if b == 7:
                emit_softmax_chunk(0, 6)

        emit_gathers(6, 7)  # b6's flat4 is final; runs during b7's tail MMs
        emit_carry(len(carry))
        emit_gathers(7, 8)
        emit_softmax_chunk(6, 8, gather=False)

    if strip:
        _split_multi_waits(nc, mybir)
    return nc


def _split_multi_waits(nc, mybir):
    """Move extra semaphore waits onto standalone NoOps on the same engine.

    This walrus build encodes at most one sync-wait command per instruction,
    but Tile emits instructions with several (cross-engine RAW + WAR + DMA
    queue ordering). A NoOp carrying one wait, placed immediately before the
    instruction in the same engine's stream, is semantically identical: the
    engine's sequencer blocks on the NoOp's wait before dispatching the real
    instruction.
    """
    n = 0
    for fn in nc.m.functions:
        for blk in fn.blocks:
            insts = blk.instructions
            new = []
            changed = False
            for inst in insts:
                si = inst.sync_info
                if si is not None and si.on_wait and len(si.on_wait) > 1:
                    for w in list(si.on_wait)[:-1]:
                        n += 1
                        new.append(mybir.InstNoOp(
                            name=f"{inst.name}-sw{n}",
                            engine=inst.engine,
                            text_hint="split_wait",
                            bass_nofuse=True,
                            sync_info=mybir.SyncInfo(
                                on_wait=[w], on_update=[]),
                        ))
                    inst.sync_info = mybir.SyncInfo(
                        on_wait=[list(si.on_wait)[-1]],
                        on_update=list(si.on_update or []))
                    changed = True
                new.append(inst)
            if changed:
                blk.instructions = new


def get_nc(strip=True):
    key = ("nc", strip)
    if key not in _CACHE:
        _CACHE[key] = _build_bass(strip)
    return _CACHE[key]


def make_in_maps(hidden, encoder_outputs, mask, W_attn, b_attn, v):
    import ml_dtypes

    bf16 = ml_dtypes.bfloat16
    f8 = ml_dtypes.float8_e4m3

    W_attn = np.asarray(W_attn, dtype=np.float32)
    Wh, We = W_attn[:K], W_attn[K:]
    # wh_pack[p, kt, dt, c] = Wh[dt*128 + p, kt*128 + c]  (kt-major chunks)
    wh_pack = np.ascontiguousarray(
        Wh.reshape(KT, 128, KT, 128).transpose(1, 2, 0, 3).astype(bf16))
    # w8_pack[p, et, j, k] = 64 * We[et*256 + 2p + j, k]
    w8_pack = np.ascontiguousarray(
        (We * WSCALE).reshape(ET2, 128, 2, K).transpose(1, 0, 2, 3).astype(f8))
    b_pack = np.ascontiguousarray(
        np.asarray(b_attn, dtype=np.float32).reshape(1, K).astype(bf16))
    # v_pack[p, kt] = v[kt*128 + p]
    v_pack = np.ascontiguousarray(
        np.asarray(v, dtype=np.float32).reshape(KT, 128).T.astype(bf16))
    hidden = np.asarray(hidden, dtype=np.float32)

    in_maps = []
    for c in range(NCORES):
        sl = slice(c * BL, (c + 1) * BL)
        # hT_pack[p, dt*BL + b] = hidden[b, dt*128 + p]
        hT_pack = np.ascontiguousarray(
            hidden[sl].T.reshape(KT, 128, BL).transpose(1, 0, 2)
            .reshape(128, KT * BL).astype(bf16))
        in_maps.append({
            "encoder_outputs": np.ascontiguousarray(encoder_outputs[sl]),
            "mask": np.ascontiguousarray(np.asarray(mask[sl], dtype=np.int32)),
            "wh_pack": wh_pack,
            "w8_pack": w8_pack,
            "hT_pack": hT_pack,
            "b_pack": b_pack,
            "v_pack": v_pack,
        })
    return in_maps


def kernel(hidden, encoder_outputs, mask, W_attn, b_attn, v):
    from concourse.bass_utils import run_bass_kernel_spmd

    nc = get_nc()
    in_maps = make_in_maps(hidden, encoder_outputs, mask, W_attn, b_attn, v)
    res = run_bass_kernel_spmd(nc, in_maps, core_ids=list(range(NCORES)))
    return np.concatenate(
        [np.asarray(res.results[c]["out"], dtype=np.float32) for c in range(NCORES)],
        axis=0,
    )
